# revision 2
# baseline (speedup 1.0000x reference)
"""u_dot_v edge scoring on 8 Trainium2 NeuronCores — v2 (one-sided gather).

score[e] = dot(h[src[e]], h[dst[e]]) for 600k edges, 128-dim features.

Strategy (vs baseline's two-sided dma_gather, which is DMA-transfer bound at
~2 x 512B x 75k descriptors per core):
  - Edges globally sorted by src; 8 contiguous shards of 75k edges.
  - Only the dst side is DMA-gathered (75k descriptors/core). dma_gather's
    int16 index limit is met by gathering from per-window compact tables:
    for every 32768 consecutive edge slots the host gathers the window's
    unique dst rows into a dedicated DRAM table slice (unique count <=
    32768 by construction), so indices are window-local ranks.
  - The src side never moves per-edge through DMA. Edges are packed into
    128-edge tiles whose (sorted) src nodes span <= 32 distinct values. The
    host lays those nodes' feature columns into a per-tile window of a
    [128, T*32] h^T table (static stride-32 APs, same program on all cores).
  - Per tile: PE transposes the gathered hv tile (PSUM), ACT copies to SBUF,
    then PE computes psum[e, c] = sum_f hv[e, f] * hT_win[f, c] with the hv^T
    tile as the stationary operand (128x128) and the 32 window columns as the
    moving operand.
  - score[e] = psum[e, col(e)] is extracted on DVE: one-hot mask from
    is_equal(iota, colidx) then multiply + free-axis reduce, batched over 16
    tiles (one PSUM bank) per instruction triplet.
  - All data paths are exact fp32 byte-movers / fp32 MACs; error vs the jnp
    reference is summation-order only (same class as the fp32 baseline).
"""

import numpy as np

from concourse import bacc, mybir, tile
from concourse.bass_utils import run_bass_kernel_spmd

P = 128
N_NODES = 100000
D_FEAT = 128
N_EDGES = 600000
N_CORES = 8
CHUNK = 1024  # edges per dma_gather call (SWDGE ring limit)
TILE = 128  # edges per matvec tile
C = 24  # h^T column window per tile
GRP = 16  # tiles per DVE extraction batch (one PSUM bank: 16*32 fp32)
TILES_PER_CHUNK = CHUNK // TILE  # 8
WIN = 32768  # edge slots per gather-table window (int16 rank bound)
WIN_CHUNKS = WIN // CHUNK  # 32


# ---------------------------------------------------------------- host plan

def _pack_tiles(svals):
    """Split a src-sorted edge-index range into tiles of <=128 edges with
    <=C distinct src values. Returns list of (start, stop) into svals."""
    n = svals.shape[0]
    bounds = []
    start = 0
    while start < n:
        stop = min(start + TILE, n)
        # distinct count within [start, stop)
        d = 1 + int(np.count_nonzero(np.diff(svals[start:stop])))
        while d > C:
            # close early: find largest stop with <= C distinct (rare path)
            uniq_pos = np.nonzero(np.diff(svals[start:stop]))[0]
            # position after the C-th distinct value begins
            stop = start + int(uniq_pos[C - 1]) + 1
            d = C
        bounds.append((start, stop))
        start = stop
    return bounds


def _plan(src, dst):
    """Shard + tile-pack all edges. Returns per-core tile lists and T."""
    order = np.argsort(src, kind="stable")
    e_core = N_EDGES // N_CORES
    packed = []  # packed[c] = list of edge-id arrays (tiles)
    for c in range(N_CORES):
        eid = order[c * e_core:(c + 1) * e_core]
        svals = src[eid]
        packed.append([eid[a:b] for a, b in _pack_tiles(svals)])
    t_total = max(len(p) for p in packed)
    t_total = ((t_total + GRP - 1) // GRP) * GRP
    return packed, t_total


def _wrap_idx16(vals):
    """[CHUNK] int16 -> [P, CHUNK//16] wrapped + replicated (dma_gather
    index layout: token j lives at [j%16 + 16*g, j//16] for every group g)."""
    w = vals.reshape(CHUNK // 16, 16).T  # [16, CHUNK//16]
    return np.tile(w, (P // 16, 1))


def _build_core_inputs(h32, src, dst, packed_c, t_total):
    """Per-core data arrays for the shared static program."""
    n_slots = t_total * TILE
    n_win = (n_slots + WIN - 1) // WIN

    slots_eid = np.full(n_slots, -1, np.int64)
    slots_col = np.zeros(n_slots, np.int16)
    slots_dst = np.zeros(n_slots, np.int64)  # dst node per slot (0 for pad)
    tbl_nodes = np.zeros(t_total * C, np.int64)

    for t, eids in enumerate(packed_c):
        s = src[eids]
        uniq, inv = np.unique(s, return_inverse=True)
        assert uniq.shape[0] <= C
        tbl_nodes[t * C:t * C + uniq.shape[0]] = uniq
        lo = t * TILE
        slots_eid[lo:lo + eids.shape[0]] = eids
        slots_col[lo:lo + eids.shape[0]] = inv.astype(np.int16)
        slots_dst[lo:lo + eids.shape[0]] = dst[eids]

    # per-window compact gather tables + window-local ranks
    dtbl = np.zeros((n_win * WIN, D_FEAT), np.float32)
    slots_rank = np.zeros(n_slots, np.int16)
    for w in range(n_win):
        lo, hi = w * WIN, min((w + 1) * WIN, n_slots)
        uniq, inv = np.unique(slots_dst[lo:hi], return_inverse=True)
        assert uniq.shape[0] <= WIN
        dtbl[w * WIN:w * WIN + uniq.shape[0]] = h32[uniq]
        slots_rank[lo:hi] = inv.astype(np.int16)

    hT_tbl = np.ascontiguousarray(h32[tbl_nodes].T)  # [128, T*C]
    colidx = np.ascontiguousarray(
        slots_col.reshape(t_total, TILE).T.astype(np.float32))  # [128, T]
    idx16 = np.empty((P, n_slots // 16), np.int16)
    for k in range(n_slots // CHUNK):
        idx16[:, k * (CHUNK // 16):(k + 1) * (CHUNK // 16)] = _wrap_idx16(
            slots_rank[k * CHUNK:(k + 1) * CHUNK])
    return {
        "dtbl": dtbl,
        "hT_tbl": hT_tbl,
        "idx16": idx16,
        "colidx": colidx,
    }, slots_eid


# ------------------------------------------------------------- device build

def emit_body(tcx, outs, ins, t_total):
    nc = tcx.nc
    dtbl = ins["dtbl"]
    hT_d = ins["hT_tbl"]
    idx_d = ins["idx16"]
    col_d = ins["colidx"]
    iota_d = ins["iota"]
    eye_d = ins["eye"]
    out = outs["score"]

    n_chunks = t_total // TILES_PER_CHUNK
    n_queues = nc.num_swdge_queues

    with tcx.tile_pool(name="res", bufs=1) as res, \
         tcx.tile_pool(name="gath", bufs=4) as gpool, \
         tcx.tile_pool(name="hTg", bufs=4) as htpool, \
         tcx.tile_pool(name="pa", bufs=2, space="PSUM") as papool, \
         tcx.tile_pool(name="stage", bufs=3) as stpool, \
         tcx.tile_pool(name="pb", bufs=2, space="PSUM") as pbpool, \
         tcx.tile_pool(name="msk", bufs=2) as mpool, \
         tcx.tile_pool(name="prd", bufs=2) as prpool:
        idx_sb = res.tile([P, t_total * TILE // 16], mybir.dt.int16, tag="idx")
        col_sb = res.tile([P, t_total], mybir.dt.float32, tag="col")
        iota_sb = res.tile([P, GRP * C], mybir.dt.float32, tag="iota")
        eye_sb = res.tile([P, P], mybir.dt.float32, tag="eye")
        score_sb = res.tile([P, t_total], mybir.dt.float32, tag="score")

        nc.sync.dma_start(out=idx_sb[:], in_=idx_d[:, :])
        nc.sync.dma_start(out=col_sb[:], in_=col_d[:, :])
        nc.sync.dma_start(out=iota_sb[:], in_=iota_d[:, :])
        nc.sync.dma_start(out=eye_sb[:], in_=eye_d[:, :])

        pb = None
        hT_g = None
        for k in range(n_chunks):
            w = k // WIN_CHUNKS
            hs = dtbl[w * WIN:(w + 1) * WIN, :]

            if k % (GRP // TILES_PER_CHUNK) == 0:
                g0 = k * TILES_PER_CHUNK
                hT_g = htpool.tile([P, GRP * C], mybir.dt.float32, tag="hTg")
                nc.sync.dma_start(out=hT_g[:],
                                  in_=hT_d[:, g0 * C:(g0 + GRP) * C])

            hv = gpool.tile([P, TILES_PER_CHUNK, D_FEAT], mybir.dt.float32,
                            tag="hv")
            nc.gpsimd.dma_gather(
                hv[:, :, :], hs,
                idx_sb[:, k * (CHUNK // 16):(k + 1) * (CHUNK // 16)],
                CHUNK, CHUNK, D_FEAT, queue_num=k % n_queues)

            pa = papool.tile([P, TILES_PER_CHUNK, D_FEAT], mybir.dt.float32,
                             tag="pa")
            for i in range(TILES_PER_CHUNK):
                nc.tensor.transpose(pa[:, i, :], hv[:, i, :], eye_sb[:])
            hvT = stpool.tile([P, TILES_PER_CHUNK, D_FEAT], mybir.dt.float32,
                              tag="hvT")
            nc.scalar.copy(out=hvT[:, :, :], in_=pa[:, :, :])

            for i in range(TILES_PER_CHUNK):
                t = k * TILES_PER_CHUNK + i
                g = t % GRP
                if g == 0:
                    pb = pbpool.tile([P, GRP, C], mybir.dt.float32, tag="pb")
                nc.tensor.matmul(
                    pb[:, g, :], lhsT=hvT[:, i, :],
                    rhs=hT_g[:, g * C:(g + 1) * C],
                    start=True, stop=True)
                if g == GRP - 1:
                    g0 = t - (GRP - 1)
                    mask = mpool.tile([P, GRP, C], mybir.dt.float32,
                                      tag="mask")
                    cb = col_sb[:, g0:g0 + GRP].unsqueeze(2).broadcast_to(
                        [P, GRP, C])
                    nc.vector.tensor_tensor(
                        out=mask[:, :, :],
                        in0=iota_sb[:].rearrange("p (g c) -> p g c", c=C),
                        in1=cb, op=mybir.AluOpType.is_equal)
                    prod = prpool.tile([P, GRP, C], mybir.dt.float32,
                                       tag="prod")
                    nc.vector.tensor_tensor(
                        out=prod[:, :, :], in0=pb[:, :, :], in1=mask[:, :, :],
                        op=mybir.AluOpType.mult)
                    nc.vector.tensor_reduce(
                        out=score_sb[:, g0:g0 + GRP], in_=prod[:, :, :],
                        axis=mybir.AxisListType.X, op=mybir.AluOpType.add)

        nc.sync.dma_start(out=out[:, :], in_=score_sb[:])


def _build(t_total):
    n_win = (t_total * TILE + WIN - 1) // WIN
    nc = bacc.Bacc("TRN2", target_bir_lowering=False, debug=False,
                   enable_asserts=False, num_swdge_queues=4)
    dtbl = nc.dram_tensor("dtbl", [n_win * WIN, D_FEAT], mybir.dt.float32,
                          kind="ExternalInput").ap()
    hT = nc.dram_tensor("hT_tbl", [P, t_total * C], mybir.dt.float32,
                        kind="ExternalInput").ap()
    idx = nc.dram_tensor("idx16", [P, t_total * TILE // 16], mybir.dt.int16,
                         kind="ExternalInput").ap()
    col = nc.dram_tensor("colidx", [P, t_total], mybir.dt.float32,
                         kind="ExternalInput").ap()
    iota = nc.dram_tensor("iota", [P, GRP * C], mybir.dt.float32,
                          kind="ExternalInput").ap()
    eye = nc.dram_tensor("eye", [P, P], mybir.dt.float32,
                         kind="ExternalInput").ap()
    out = nc.dram_tensor("score", [P, t_total], mybir.dt.float32,
                         kind="ExternalOutput").ap()
    with tile.TileContext(nc) as tcx:
        emit_body(tcx, {"score": out},
                  {"dtbl": dtbl, "hT_tbl": hT, "idx16": idx, "colidx": col,
                   "iota": iota, "eye": eye}, t_total)
    nc.compile()
    return nc


# -------------------------------------------------------------------- run

def _prepare(h, src, dst):
    h32 = np.ascontiguousarray(np.asarray(h, dtype=np.float32))
    src = np.asarray(src).astype(np.int64)
    dst = np.asarray(dst).astype(np.int64)
    packed, t_total = _plan(src, dst)

    iota_const = np.ascontiguousarray(
        np.broadcast_to(np.tile(np.arange(C, dtype=np.float32), GRP),
                        (P, GRP * C)))
    eye_const = np.eye(P, dtype=np.float32)

    in_maps, slot_maps = [], []
    for c in range(N_CORES):
        m, slots_eid = _build_core_inputs(h32, src, dst, packed[c], t_total)
        m["iota"] = iota_const
        m["eye"] = eye_const
        in_maps.append(m)
        slot_maps.append(slots_eid)
    return in_maps, slot_maps, t_total


def _gather_out(results, slot_maps, t_total):
    out = np.empty((N_EDGES, 1), np.float32)
    for c in range(N_CORES):
        sc = results[c]["score"]  # [P, T]
        flat = sc.T.reshape(-1)  # slot t*128+p
        eid = slot_maps[c]
        valid = eid >= 0
        out[eid[valid], 0] = flat[valid]
    return out


def _run(h, src, dst, trace=False, **run_kwargs):
    in_maps, slot_maps, t_total = _prepare(h, src, dst)
    nc = _build(t_total)
    res = run_bass_kernel_spmd(nc, in_maps, core_ids=list(range(N_CORES)),
                               trace=trace, **run_kwargs)
    return _gather_out(res.results, slot_maps, t_total), res


def kernel(h, src, dst):
    out, _ = _run(h, src, dst)
    return out
